# revision 40
# baseline (speedup 1.0000x reference)
"""LocalWindowAttention Trainium2 kernel (Bass/Tile), 8-core SPMD.

Problem: x[B=4, S=4096, E=512] -> out[B, S, E]
  qkv = x @ W_qkv + b_qkv ; q,k,v = split(qkv)
  scores = (q @ k.T) / sqrt(E), banded mask |i-j| <= 64, softmax
  out = (attn @ v) @ W_out + b_out

Sharding: 8 cores = (batch b in 0..3) x (seq half h in 0..1). Each core owns
2048 query rows and loads a 64-row halo of x on each side (zero-padded at
sequence boundaries), computing q/k/v locally - no collectives.

Key structural choices:
  - W_out is folded into the v-projection on the host:
      (attn @ v) @ W_out = attn @ (x @ (W_v @ W_out))
    so the output projection disappears from the kernel. Since attention
    rows sum to 1, the output bias (b_v @ W_out + b_out) is folded into
    the v rows themselves (v'' = v' + b_vo added during the PSUM->SBUF
    copy), which makes softmax normalization a pure per-partition scale.
  - All matmul operands are bf16 (1 cycle/row at any moving size, FWL
    weight loads, half the DMA bytes). PSUM accumulation stays fp32.
  - Scores are computed TRANSPOSED, [key, query], with k-chunks as the
    stationary operand: the exp output is directly the stationary operand
    of the attended matmul -> no PE transposes at all.
  - The band mask is MULTIPLICATIVE (0/1 bf16) applied after exp as one
    DVE op per tile pair (raw scores are O(1) so unmasked exp is safe).
  - Row sums for softmax come from a ones-column appended to the v tiles
    (attended matmul emits [q, 256 feats + rowsum] per half); the final
    normalize is scalar-engine activation with per-partition scale 1/rowsum.
  - Inputs stream on two HW DMA queues (SP: xT + output, ACT: weights),
    ordered so the PE starts in ~5us and never starves.
"""

import sys

sys.path.insert(0, "/opt/trn_rl_repo")

import numpy as np
import ml_dtypes

import concourse.bass as bass  # noqa: F401  (registers types)
import concourse.tile as tile
from concourse import bacc, mybir
from concourse.bass_utils import run_bass_kernel_spmd

F32 = mybir.dt.float32
BF16 = mybir.dt.bfloat16
BF16_NP = ml_dtypes.bfloat16

B, S, E = 4, 4096, 512
WINDOW = 64
HALF = S // 2              # 2048 query rows per core
ROWS = HALF + 2 * WINDOW   # 2176 local rows incl. halo
NT = HALF // 128           # 16 query subtiles per core
NCH = ROWS // 128          # 17 v chunks

# qT matmul groups in xT col space (queries live at local rows [64, 2112))
QSLC = [(64, 192), (256, 512), (768, 512), (1280, 512), (1792, 320)]
# kT matmul groups (full local rows)
KSLC = [(0, 256), (256, 512), (768, 512), (1280, 512), (1792, 384)]

_NC_CACHE = {}


def _build():
    nc = bacc.Bacc("TRN2", target_bir_lowering=False, debug=False, num_devices=8)

    xT_d = nc.dram_tensor("xT", [E, ROWS], BF16, kind="ExternalInput")
    wqkv_d = nc.dram_tensor("wqkv", [E, 3 * E], BF16, kind="ExternalInput")
    bqk_d = nc.dram_tensor("bqk", [128, 8], F32, kind="ExternalInput")
    mask_d = nc.dram_tensor("masks", [128, 1536], BF16, kind="ExternalInput")
    brep_d = nc.dram_tensor("brep", [128, E], F32, kind="ExternalInput")
    out_d = nc.dram_tensor("out", [HALF, E], BF16, kind="ExternalOutput")

    ACT = mybir.ActivationFunctionType
    ALU = mybir.AluOpType

    with tile.TileContext(nc) as tc:
        with (
            tc.tile_pool(name="const", bufs=1) as const,
            tc.tile_pool(name="big", bufs=1) as big,
        ):
            # ---- constants ----
            wq_sb = [const.tile([128, 3 * E], BF16, name=f"wq{e}", tag=f"wq{e}")
                     for e in range(4)]
            bqk_sb = const.tile([128, 8], F32, name="bqk", tag="bqk")
            mask_sb = const.tile([128, 12, 128], BF16, name="msk", tag="msk")
            brep_sb = const.tile([128, 2, 256], F32, name="brep", tag="brep")

            # ---- persistent products ----
            qT = [big.tile([128, HALF], BF16, name=f"qT{f}", tag=f"qT{f}")
                  for f in range(4)]
            kT = [big.tile([128, ROWS], BF16, name=f"kT{f}", tag=f"kT{f}")
                  for f in range(4)]
            # v rows with W_out and output bias folded in; per 128-row chunk:
            # [h, 257] where col 256 of each half is 1.0 (rowsum column)
            vaug = [big.tile([128, 2, 257], BF16, name=f"v{r}", tag=f"v{r}")
                    for r in range(NCH)]

            xTp = [big.tile([128, ROWS], BF16, name=f"xT{e}", tag=f"xT{e}")
                   for e in range(4)]

            # ones columns for the rowsum trick (off critical path)
            for r in range(NCH):
                nc.gpsimd.memset(vaug[r][:, :, 256:257], 1.0)

            # PE warmup: dummy matmuls on zeroed data run during the input
            # DMA wait, so the HAM clock gate reaches 8/8 before real work
            # (cold PE runs at 1.2 GHz for the first ~3.4us otherwise).
            warm = const.tile([128, 256], BF16, name="warm", tag="warm")
            nc.vector.memset(warm[:, :], 0.0)
            with tc.tile_pool(name="pw", bufs=1, space="PSUM") as pw:
                wps = pw.tile([128, 128], F32, name="wps", tag="wps")
                for _ in range(40):
                    nc.tensor.matmul(wps[:, :], warm[:, 0:128],
                                     warm[:, 128:256], start=True, stop=True)

            # ---- input DMAs ----
            # ACT queue: biases, then weight chunks interleaved in the order
            # the projection groups consume them. SP queue: xT slice-major.
            nc.scalar.dma_start(out=bqk_sb, in_=bqk_d[:, :])
            for e in range(4):
                nc.sync.dma_start(out=xTp[e][:, 0:320],
                                  in_=xT_d[128 * e:128 * (e + 1), 0:320])
            for e in range(4):
                nc.scalar.dma_start(out=wq_sb[e][:, 0:256],
                                    in_=wqkv_d[128 * e:128 * (e + 1), 0:256])
            for e in range(4):
                nc.sync.dma_start(out=xTp[e][:, 320:832],
                                  in_=xT_d[128 * e:128 * (e + 1), 320:832])
            for e in range(4):
                nc.scalar.dma_start(out=wq_sb[e][:, 256:E],
                                    in_=wqkv_d[128 * e:128 * (e + 1), 256:E])
            for e in range(4):
                nc.scalar.dma_start(out=wq_sb[e][:, E:2 * E],
                                    in_=wqkv_d[128 * e:128 * (e + 1), E:2 * E])
            for e in range(4):
                nc.sync.dma_start(out=xTp[e][:, 832:1280],
                                  in_=xT_d[128 * e:128 * (e + 1), 832:1280])
            for e in range(4):
                nc.sync.dma_start(out=xTp[e][:, 1280:ROWS],
                                  in_=xT_d[128 * e:128 * (e + 1), 1280:ROWS])
            for e in range(4):
                nc.scalar.dma_start(out=wq_sb[e][:, 2 * E:3 * E],
                                    in_=wqkv_d[128 * e:128 * (e + 1), 2 * E:3 * E])
            nc.scalar.dma_start(out=mask_sb[:, :, :], in_=mask_d[:, :])
            nc.scalar.dma_start(out=brep_sb[:, :, :], in_=brep_d[:, :])

            # ---- q/k projections ----
            # Slice-major over xT columns so compute starts as soon as the
            # first slice lands; the first two qT groups run while the
            # k-weights are still streaming in.
            with tc.tile_pool(name="pp", bufs=4, space="PSUM") as pp:
                def qT_group(si):
                    q0, qn = QSLC[si]
                    for f in range(4):
                        ps = pp.tile([128, 512], F32,
                                     name=f"pq{f}_{si}", tag="pp")
                        for e in range(4):
                            nc.tensor.matmul(
                                ps[:, :qn],
                                wq_sb[e][:, 128 * f:128 * (f + 1)],
                                xTp[e][:, q0:q0 + qn],
                                start=(e == 0), stop=(e == 3),
                            )
                        nc.vector.tensor_scalar_add(
                            qT[f][:, q0 - 64:q0 - 64 + qn], ps[:, :qn],
                            bqk_sb[:, f:f + 1],
                        )

                def kT_group(si):
                    k0, kn = KSLC[si]
                    for f in range(4):
                        ps = pp.tile([128, 512], F32,
                                     name=f"pk{f}_{si}", tag="pp")
                        for e in range(4):
                            nc.tensor.matmul(
                                ps[:, :kn],
                                wq_sb[e][:, E + 128 * f:E + 128 * (f + 1)],
                                xTp[e][:, k0:k0 + kn],
                                start=(e == 0), stop=(e == 3),
                            )
                        nc.scalar.activation(
                            out=kT[f][:, k0:k0 + kn], in_=ps[:, :kn],
                            func=ACT.Identity, bias=bqk_sb[:, 4 + f:5 + f],
                        )

                # q-weights land ~4us before k-weights: run the first two
                # qT groups while wk streams in
                qT_group(0)
                qT_group(1)
                kT_group(0)
                kT_group(1)
                for si in range(2, 5):
                    qT_group(si)
                    kT_group(si)

            # ---- attention tiles, processed in pairs ----
            # Tiles (2P, 2P+1) share key chunk 2P+1, so a pair needs 3 key
            # chunks: c0=2P (queries 2P only), c1=2P+1 (both query tiles,
            # N=256), c2=2P+2 (queries 2P+1 only) -> 12 score matmuls per
            # pair instead of 16, one packed PSUM bank [128,4,128], one exp
            # and one mask op per pair. ps_a is declared first so it reuses
            # the projection pool's banks; ps_s lands on fresh banks and
            # hoisted score matmuls never WAR-serialize against projections.
            with (
                tc.tile_pool(name="attn", bufs=4) as attn,
                tc.tile_pool(name="pv", bufs=2, space="PSUM") as pv,
                tc.tile_pool(name="ps_a", bufs=4, space="PSUM") as ps_a,
                tc.tile_pool(name="ps_s", bufs=2, space="PSUM") as ps_s,
            ):
                def v_chunks(r0, r1):
                    # v'' = x @ (W_v @ W_out) + (b_v @ W_out + b_out),
                    # natural [rows, feat] layout (bias fused into the copy)
                    for r in range(r0, r1):
                        ps = pv.tile([128, 2, 256], F32, name=f"pv{r}", tag="pv")
                        for e in range(4):
                            nc.tensor.matmul(
                                ps[:, :, :],
                                xTp[e][:, 128 * r:128 * (r + 1)],
                                wq_sb[e][:, 2 * E:3 * E],
                                start=(e == 0), stop=(e == 3),
                            )
                        nc.vector.tensor_add(
                            vaug[r][:, :, 0:256], ps[:, :, :], brep_sb[:, :, :])

                def pair_scores(P):
                    t0, t1 = 2 * P, 2 * P + 1
                    ps4 = ps_s.tile([128, 4, 128], F32, name=f"s{P}", tag="ps_s")
                    for f in range(4):
                        nc.tensor.matmul(
                            ps4[:, 0:1, :],
                            kT[f][:, 128 * t0:128 * (t0 + 1)],
                            qT[f][:, 128 * t0:128 * (t0 + 1)],
                            start=(f == 0), stop=(f == 3),
                        )
                    for f in range(4):
                        nc.tensor.matmul(
                            ps4[:, 1:3, :],
                            kT[f][:, 128 * t1:128 * (t1 + 1)],
                            qT[f][:, 128 * t0:128 * (t0 + 2)],
                            start=(f == 0), stop=(f == 3),
                        )
                    for f in range(4):
                        nc.tensor.matmul(
                            ps4[:, 3:4, :],
                            kT[f][:, 128 * (t1 + 1):128 * (t1 + 2)],
                            qT[f][:, 128 * t1:128 * (t1 + 1)],
                            start=(f == 0), stop=(f == 3),
                        )
                    # exp (raw scores are O(1)), then one multiplicative 0/1
                    # band-mask op for the whole pair
                    mi = 1 if P == 0 else (2 if P == NT // 2 - 1 else 0)
                    ept = attn.tile([128, 4, 128], BF16, name=f"pe{P}", tag="pe")
                    if P == NT // 2 - 1:
                        # final pair: per-half exp/mask so the drain chain
                        # (nothing left to overlap with) is shorter
                        for hb in range(2):
                            nc.scalar.activation(
                                out=ept[:, 2 * hb:2 * hb + 2, :],
                                in_=ps4[:, 2 * hb:2 * hb + 2, :], func=ACT.Exp)
                            nc.vector.tensor_tensor(
                                ept[:, 2 * hb:2 * hb + 2, :],
                                ept[:, 2 * hb:2 * hb + 2, :],
                                mask_sb[:, 4 * mi + 2 * hb:4 * mi + 2 * hb + 2, :],
                                op=ALU.mult)
                    else:
                        nc.scalar.activation(
                            out=ept[:, :, :], in_=ps4[:, :, :], func=ACT.Exp)
                        nc.vector.tensor_tensor(
                            ept[:, :, :], ept[:, :, :],
                            mask_sb[:, 4 * mi:4 * mi + 4, :], op=ALU.mult)
                    return ept

                def pair_attended(P, ept):
                    t0, t1 = 2 * P, 2 * P + 1
                    # attended (unnormalized) + rowsum via the ones column
                    for ti, t in ((0, t0), (1, t1)):
                        paA = ps_a.tile([128, 257], F32,
                                        name=f"paA{t}", tag="ps_a")
                        paB = ps_a.tile([128, 257], F32,
                                        name=f"paB{t}", tag="ps_a")
                        for kc in range(2):
                            eslc = ept[:, 2 * ti + kc:2 * ti + kc + 1, :]
                            nc.tensor.matmul(
                                paA[:, :], eslc, vaug[t + kc][:, 0:1, :],
                                start=(kc == 0), stop=(kc == 1),
                            )
                            nc.tensor.matmul(
                                paB[:, :], eslc, vaug[t + kc][:, 1:2, :],
                                start=(kc == 0), stop=(kc == 1),
                            )
                        rd = attn.tile([128, 1], F32, name=f"rd{t}", tag="rd")
                        nc.vector.reciprocal(rd[:], paA[:, 256:257])
                        # out = attended * (1/rowsum); bias already in v rows
                        ost = attn.tile([128, 512], BF16,
                                        name=f"ost{t}", tag="ost")
                        nc.scalar.activation(
                            out=ost[:, 0:256], in_=paA[:, 0:256],
                            func=ACT.Identity, scale=rd[:])
                        nc.vector.tensor_scalar_mul(
                            ost[:, 256:512], paB[:, 0:256], rd[:])
                        nc.sync.dma_start(
                            out=out_d[128 * t:128 * (t + 1), :], in_=ost[:])

                def pair(P):
                    pair_attended(P, pair_scores(P))

                # v-chunk projections interleave with attention pairs: the
                # v matmuls fill the PE while each pair's exp/mask/normalize
                # chain runs on the other engines. The final pair's scores
                # (which need no v chunks) run early so the end-of-kernel
                # drain is only its attended+normalize chain.
                v_chunks(0, 3)
                pair(0)
                v_chunks(3, 5)
                pair(1)
                v_chunks(5, 7)
                pair(2)
                v_chunks(7, 9)
                pair(3)
                v_chunks(9, 11)
                pair(4)
                v_chunks(11, 13)
                pair(5)
                v_chunks(13, 15)
                e7 = pair_scores(7)
                pair(6)
                v_chunks(15, 17)
                pair_attended(7, e7)
    nc.compile()
    return nc


def _get_nc():
    if "nc" not in _NC_CACHE:
        _NC_CACHE["nc"] = _build()
    return _NC_CACHE["nc"]


def _prep_shared(W_qkv, b_qkv, W_out, b_out):
    scale = 1.0 / np.sqrt(np.float64(E))
    W = np.array(W_qkv, dtype=np.float64)
    Wo = np.array(W_out, dtype=np.float64)
    b = np.array(b_qkv, dtype=np.float64)
    bo = np.array(b_out, dtype=np.float64)

    wq = W[:, :E] * scale
    wk = W[:, E:2 * E]
    wvo = W[:, 2 * E:3 * E] @ Wo          # fold output projection into v
    wqkv = np.concatenate([wq, wk, wvo], axis=1)

    bq = b[:E] * scale
    bk = b[E:2 * E]
    bqk = np.stack([*(bq.reshape(4, 128)), *(bk.reshape(4, 128))], axis=1)
    bvo = b[2 * E:3 * E] @ Wo + bo        # folded output bias

    shared = {
        "wqkv": np.ascontiguousarray(wqkv.astype(np.float32)).astype(BF16_NP),
        "bqk": np.ascontiguousarray(bqk.astype(np.float32)),
        "brep": np.ascontiguousarray(
            np.tile(bvo.astype(np.float32)[None, :], (128, 1))),
    }
    return shared


def _masks_for(h: int) -> np.ndarray:
    """Multiplicative 0/1 masks in TRANSPOSED [key-in-chunk, block, query]
    layout for PAIRED tiles. Blocks per pair: [c0 | c1(for t0) | c1(for t1)
    | c2]; c0/c1(t0) see the key chunk as window-low (upper-triangular),
    c1(t1)/c2 as window-high (lower-triangular). Variants along dim1:
    [interior | first-pair | last-pair]."""
    j = np.arange(128)[:, None]           # key index within chunk
    i = np.arange(128)[None, :]           # query index within tile
    ut = (j - i >= 0)                     # key chunk == query tile: jj-i in [0,128]
    lt = (j <= i)                         # key chunk one above: jj-i in [0,128]
    ut_e = ut & (j >= 64) if h == 0 else ut       # seq start: key >= 0
    lt_e = lt & (j < 64) if h == 1 else lt        # seq end: key < S
    interior = np.stack([ut, lt, ut, lt], axis=1)
    first = np.stack([ut_e, lt, ut, lt], axis=1)
    last = np.stack([ut, lt, ut, lt_e], axis=1)
    stacked = np.concatenate([interior, first, last], axis=1)  # [128, 12, 128]
    return np.ascontiguousarray(
        stacked.reshape(128, 1536).astype(np.float32)).astype(BF16_NP)


def _install_ntff_shim():
    """The agent image's antenv lacks axon_hooks; synthesize it from the
    boot module's ctypes NTFF driver so trace=True can capture HW timing."""
    import types
    if "antenv.axon_hooks" in sys.modules:
        return
    try:
        from trn_agent_boot.trn_boot import _ntff_profile_via_ctypes
        hook = _ntff_profile_via_ctypes("/opt/axon/libaxon_pjrt.so")
    except Exception:
        hook = None
    mod = types.ModuleType("antenv.axon_hooks")
    mod.get_axon_ntff_profile_hook = lambda: hook
    mod.set_axon_ntff_profile_hook = lambda h: None
    sys.modules["antenv.axon_hooks"] = mod
    # avoid S3 artifact upload attempts during local profile processing
    try:
        from concourse import bass_utils as _bu
        _bu.upload_artifacts = lambda tmpdir: tmpdir
    except Exception:
        pass


def kernel(x, W_qkv, b_qkv, W_out, b_out, _trace=False):
    x = np.asarray(x, dtype=np.float32)
    nc = _get_nc()
    shared = _prep_shared(W_qkv, b_qkv, W_out, b_out)
    masks = [_masks_for(0), _masks_for(1)]

    in_maps = []
    for core in range(8):
        b, h = divmod(core, 2)
        lo = h * HALF - WINDOW
        hi = lo + ROWS
        xh = np.zeros((ROWS, E), dtype=np.float32)
        s0, s1 = max(lo, 0), min(hi, S)
        xh[s0 - lo:s1 - lo] = x[b, s0:s1]
        in_maps.append({
            "xT": np.ascontiguousarray(xh.T).astype(BF16_NP),
            "masks": masks[h],
            **shared,
        })

    kwargs = {}
    if _trace:
        _install_ntff_shim()
        kwargs = dict(trace=True, trace_cores=[0])
    res = run_bass_kernel_spmd(nc, in_maps, core_ids=list(range(8)), **kwargs)

    out = np.empty((B, S, E), dtype=np.float32)
    for core in range(8):
        b, h = divmod(core, 2)
        out[b, h * HALF:(h + 1) * HALF] = res.results[core]["out"].astype(np.float32)
    if _trace:
        return out, res
    return out
